# revision 2
# baseline (speedup 1.0000x reference)
"""BiMamba block Trainium2 kernel v2 — 8 NeuronCores.

Sharding: core c = 4*b + 2*dir + half (b batch, dir direction, half of
d_inner).  rank = c % 4 = 2*dir + half also indexes the forward-time
L-quarter this core owns after the y-combine ReduceScatter.

- in_proj / out_proj in fp8 (e4m3) with DoubleRow matmuls.
- Scan n-accumulation via identity-matmul into PSUM (iden carries S_YG,
  and an extra u*D idmm term folds the skip connection in); scan ops on
  Vector (only engine with the scan), multiplies split Vector/GpSimd.
- Direction combine via ONE quad ReduceScatter over forward-time
  quarters: each core evacuates its out_proj partial twice (masked by
  per-core fwd/rev inputs); the reversed copy is DMA-accumulated with a
  negative-stride access pattern, so dir=1 contributions land
  time-flipped with an SPMD-uniform program.
- FFN: within each L-half pair, cores split d_ff in half and share the
  L-half (AllGather-pair of x2 quarters), partials combined with a
  ReduceScatter-pair; final quad AllGather assembles outT.
"""

import numpy as np
import ml_dtypes

import concourse.bass as bass
import concourse.bacc as bacc
import concourse.mybir as mybir
import concourse.tile as tile
from contextlib import ExitStack

F32 = mybir.dt.float32
F16 = mybir.dt.float16
BF16 = mybir.dt.bfloat16
FP8 = mybir.dt.float8e4
Alu = mybir.AluOpType
Act = mybir.ActivationFunctionType
PerfMode = mybir.MatmulPerfMode

P = 128
DM = 1024
L = 1024
DIH = 1024
NST = 16
DTR = 64
KT = DM // P        # 8
DT = DIH // P       # 8
LQ = L // 4         # 256
LH = L // 2         # 512
DFH = 2048          # d_ff half
EPS = 1e-5

S_X = 32.0          # xnorm fp8 scale
S_W = 1024.0        # in_proj weight fp8 scale
S_YG = 128.0        # gated-y fp8 scale
S_O = 2048.0        # out_proj weight fp8 scale

_BF = ml_dtypes.bfloat16
_F8 = ml_dtypes.float8_e4m3

PAIRS = [[0, 1], [2, 3], [4, 5], [6, 7]]
QUADS = [[0, 1, 2, 3], [4, 5, 6, 7]]

# Elementwise multiplies in the scan: op j goes to Vector when
# (j % 10) < MULT_VEC_OF_10, else GpSimd.  GpSimd shares its SBUF port
# with the DVE, so offloading there slows the scan ops ~1.6x — keep
# everything on Vector.
MULT_VEC_OF_10 = 10


def build_program():
    nc = bacc.Bacc("TRN2", target_bir_lowering=False, debug=False,
                   num_devices=8)

    # ---- I/O ----
    xT_ln = nc.dram_tensor("xT_ln", [DM, L], BF16, kind="ExternalInput")
    x_res = nc.dram_tensor("x_res", [DM, LQ], F32, kind="ExternalInput")
    win8 = nc.dram_tensor("win8", [DM, 2 * DIH], FP8, kind="ExternalInput")
    win_b = nc.dram_tensor("win_b", [2 * DIH], F32, kind="ExternalInput")
    sinv = nc.dram_tensor("sinv", [P], F32, kind="ExternalInput")
    conv_w = nc.dram_tensor("conv_w", [DIH, 4], F32, kind="ExternalInput")
    conv_b = nc.dram_tensor("conv_b", [DIH], F32, kind="ExternalInput")
    a_mat = nc.dram_tensor("a_mat", [DIH, NST], F32, kind="ExternalInput")
    xpw_t = nc.dram_tensor("xpw_t", [DIH, 96], BF16, kind="ExternalInput")
    dtw_t = nc.dram_tensor("dtw_t", [DTR, DIH], BF16, kind="ExternalInput")
    dt_b = nc.dram_tensor("dt_b", [DIH], F32, kind="ExternalInput")
    d_par = nc.dram_tensor("d_par", [DIH], F32, kind="ExternalInput")
    outw8 = nc.dram_tensor("outw8", [DIH, DM], FP8, kind="ExternalInput")
    mf_in = nc.dram_tensor("mf_in", [P], F32, kind="ExternalInput")
    mr_in = nc.dram_tensor("mr_in", [P], F32, kind="ExternalInput")
    iden_in = nc.dram_tensor("iden_in", [P, P], F16, kind="ExternalInput")
    w1h_t = nc.dram_tensor("w1h_t", [DM, DFH], BF16, kind="ExternalInput")
    b1_h = nc.dram_tensor("b1_h", [DFH], F32, kind="ExternalInput")
    w2h_t = nc.dram_tensor("w2h_t", [DFH, DM], BF16, kind="ExternalInput")
    b2_e = nc.dram_tensor("b2_e", [DM], F32, kind="ExternalInput")
    outT = nc.dram_tensor("outT", [4, DM, LQ], BF16, kind="ExternalOutput")

    def vec_pt(dram_vec, pool, dt_, tag):
        t = pool.tile([P, dram_vec.shape[0] // P], dt_, tag=tag, name=tag)
        nc.sync.dma_start(t[:], dram_vec.rearrange("(o p) -> p o", p=P))
        return t

    def col_pt(dram_vec, pool, tag):
        t = pool.tile([P, 1], F32, tag=tag, name=tag)
        nc.sync.dma_start(t[:], dram_vec.rearrange("(p o) -> p o", o=1))
        return t

    with tile.TileContext(nc) as tc, ExitStack() as es:
        pc = es.enter_context(tc.tile_pool(name="const", bufs=1))
        psum = es.enter_context(tc.tile_pool(name="psum", bufs=2,
                                             space="PSUM"))
        scr = es.enter_context(tc.tile_pool(name="scr", bufs=3))
        statp = es.enter_context(tc.tile_pool(name="statp", bufs=3))
        dram = es.enter_context(tc.tile_pool(name="dram", bufs=1,
                                             space="DRAM"))

        # constants
        ones_bf = pc.tile([P, 1], BF16, tag="onesb")
        nc.vector.memset(ones_bf[:], 1.0)
        wbv = vec_pt(win_b, pc, F32, "wbv")
        sinv_c = col_pt(sinv, pc, "sinv")
        cw = pc.tile([P, DT, 4], F32, tag="cw")
        nc.sync.dma_start(cw[:], conv_w.rearrange("(o p) k -> p o k", p=P))
        cb = vec_pt(conv_b, pc, F32, "cb")
        a_sb = pc.tile([P, DT, NST], F32, tag="a")
        nc.sync.dma_start(a_sb[:], a_mat.rearrange("(o p) n -> p o n", p=P))
        dtb = vec_pt(dt_b, pc, F32, "dtb")
        dpv = vec_pt(d_par, pc, F32, "dpv")
        mf_c = col_pt(mf_in, pc, "mfc")
        mr_c = col_pt(mr_in, pc, "mrc")
        iden = pc.tile([P, P], F16, tag="iden")
        nc.sync.dma_start(iden[:], iden_in[:])
        b1s = vec_pt(b1_h, pc, F32, "b1s")
        b2s = vec_pt(b2_e, pc, F32, "b2s")

        rs_in = dram.tile([4, 2, DM, LQ], BF16)
        rs_out = dram.tile([2, DM, LQ], BF16)
        agx_in = dram.tile([DM, LQ], BF16)
        agx_out = dram.tile([2, DM, LQ], BF16)
        rsf_in = dram.tile([2, DM, LQ], BF16)
        rsf_out = dram.tile([DM, LQ], BF16)
        ag_in = dram.tile([DM, LQ], BF16)
        ag_out = dram.tile([4, DM, LQ], BF16)
        xd_in = dram.tile([96, L], F32)
        xd_out = dram.tile([96, L], F32)
        bc_dram = dram.tile([2 * NST, L], F16)

        with tc.tile_pool(name="pS", bufs=1) as pS:
            # persistent through scan + out_proj
            delta = pS.tile([P, DT, L], F16, tag="delta")
            du = pS.tile([P, DT, L], F16, tag="du")
            uD = pS.tile([P, DT, L], F16, tag="uD")
            silz = pS.tile([P, DT, L], BF16, tag="silz")
            yg8 = pS.tile([P, DT, L], FP8, tag="yg8")
            outw_s = pS.tile([P, DT, DM], FP8, tag="outw")
            xdbl = pS.tile([96, L], F32, tag="xdbl")

            # ---------- Phases 0-5 (pA scope) ----------
            with tc.tile_pool(name="pA", bufs=1) as pA, \
                 tc.tile_pool(name="pAx", bufs=3) as pAx:
                win_s = pA.tile([P, KT, 2 * DIH], FP8, tag="win")
                nc.sync.dma_start(
                    win_s[:], win8.rearrange("(o p) e -> p o e", p=P))
                u_bf = pA.tile([P, DT, L], BF16, tag="u")

                xlnv = xT_ln.rearrange("(o p) t -> p o t", p=P)

                def _load_x(kt):
                    xk = pAx.tile([P, L], BF16, tag="xk", name="xk")
                    nc.sync.dma_start(xk[:], xlnv[:, kt, :])
                    return xk[:]

                # LN1 stats
                ps_m = [psum.tile([1, 512], F32, tag="st", name=f"m{h}")
                        for h in range(2)]
                ps_s = [psum.tile([1, 512], F32, tag="st2", name=f"s{h}")
                        for h in range(2)]
                for kt in range(KT):
                    xk = _load_x(kt)
                    sq = scr.tile([P, L], BF16, tag="big", name="sq")
                    nc.scalar.square(sq[:], xk)
                    for h in range(2):
                        sl = slice(h * 512, (h + 1) * 512)
                        nc.tensor.matmul(ps_m[h][:], ones_bf[:], xk[:, sl],
                                         start=(kt == 0),
                                         stop=(kt == KT - 1))
                        nc.tensor.matmul(ps_s[h][:], ones_bf[:], sq[:, sl],
                                         start=(kt == 0),
                                         stop=(kt == KT - 1))
                mean1 = statp.tile([1, L], F32, tag="stat", name="mean1")
                var1 = statp.tile([1, L], F32, tag="stat", name="var1")
                m2 = statp.tile([1, L], F32, tag="stat", name="m2")
                for h in range(2):
                    sl = slice(h * 512, (h + 1) * 512)
                    nc.vector.tensor_scalar_mul(mean1[:, sl], ps_m[h][:],
                                                1.0 / DM)
                    nc.vector.tensor_scalar_mul(var1[:, sl], ps_s[h][:],
                                                1.0 / DM)
                nc.vector.tensor_tensor(m2[:], mean1[:], mean1[:], Alu.mult)
                nc.vector.tensor_tensor(var1[:], var1[:], m2[:],
                                        Alu.subtract)
                nc.vector.tensor_scalar_add(var1[:], var1[:], EPS)
                nc.scalar.sqrt(var1[:], var1[:])
                nc.vector.reciprocal(m2[:], var1[:])
                nc.vector.tensor_scalar_mul(m2[:], m2[:], S_X)
                mean_bc = pA.tile([P, L], F32, tag="meanbc")
                rstd_bc = pA.tile([P, L], F32, tag="rstdbc")
                nc.gpsimd.partition_broadcast(mean_bc[:], mean1[:])
                nc.gpsimd.partition_broadcast(rstd_bc[:], m2[:])

                # LN1 apply -> fp8 (scaled by S_X via rstd)
                xn8 = pA.tile([P, KT, L], FP8, tag="xn8")
                for kt in range(KT):
                    xk = _load_x(kt)
                    t1 = scr.tile([P, L], F32, tag="big", name="t1")
                    nc.vector.tensor_tensor(t1[:], xk, mean_bc[:],
                                            Alu.subtract)
                    t2 = scr.tile([P, L], BF16, tag="big", name="t2")
                    nc.vector.tensor_tensor(t2[:], t1[:], rstd_bc[:],
                                            Alu.mult)
                    nc.scalar.copy(xn8[:, kt, :], t2[:])

                # in_proj (fp8 DoubleRow)
                xiT = pA.tile([P, DT, L + 3], BF16, tag="xi")
                nc.vector.memset(xiT[:, :, 0:3], 0.0)
                for m in range(2 * DT):
                    pss = [psum.tile([P, 512], F32, tag="mm",
                                     name=f"pss{h}") for h in range(2)]
                    for kp in range(KT // 2):
                        for h in range(2):
                            nc.tensor.matmul(
                                pss[h][:],
                                win_s[:, 2 * kp:2 * kp + 2,
                                      m * P:(m + 1) * P],
                                xn8[:, 2 * kp:2 * kp + 2,
                                    h * 512:(h + 1) * 512],
                                start=(kp == 0), stop=(kp == KT // 2 - 1),
                                perf_mode=PerfMode.DoubleRow)
                    for h in range(2):
                        if m < DT:
                            nc.scalar.activation(
                                xiT[:, m, 3 + h * 512:3 + (h + 1) * 512],
                                pss[h][:], Act.Identity,
                                bias=wbv[:, m:m + 1], scale=sinv_c[:, 0:1])
                        else:
                            nc.scalar.activation(
                                silz[:, m - DT, h * 512:(h + 1) * 512],
                                pss[h][:], Act.Silu,
                                bias=wbv[:, m:m + 1], scale=sinv_c[:, 0:1])

                # conv + silu
                for d in range(DT):
                    acc = scr.tile([P, L], BF16, tag="big", name="acc")
                    nc.vector.tensor_scalar(
                        acc[:], xiT[:, d, 0:L], cw[:, d, 0:1],
                        cb[:, d:d + 1], Alu.mult, Alu.add)
                    for k in (1, 2, 3):
                        nc.vector.scalar_tensor_tensor(
                            acc[:], xiT[:, d, k:k + L], cw[:, d, k:k + 1],
                            acc[:], Alu.mult, Alu.add)
                    nc.scalar.activation(u_bf[:, d, :], acc[:], Act.Silu)

                # x_proj + pair AllReduce
                xpw = pA.tile([P, DT, 96], BF16, tag="xpw")
                nc.sync.dma_start(
                    xpw[:], xpw_t.rearrange("(o p) r -> p o r", p=P))
                psx = [psum.tile([96, 512], F32, tag="mm", name=f"px{h}")
                       for h in range(2)]
                for d in range(DT):
                    for h in range(2):
                        nc.tensor.matmul(
                            psx[h][:], xpw[:, d, :],
                            u_bf[:, d, h * 512:(h + 1) * 512],
                            start=(d == 0), stop=(d == DT - 1))
                xdp = scr.tile([P, L], F32, tag="big", name="xdp")
                for h in range(2):
                    nc.scalar.copy(xdp[0:96, h * 512:(h + 1) * 512],
                                   psx[h][:])
                nc.sync.dma_start(xd_in[:], xdp[0:96, :])
                nc.gpsimd.collective_compute(
                    "AllReduce", Alu.add, PAIRS,
                    ins=[xd_in.opt()], outs=[xd_out.opt()])
                nc.sync.dma_start(xdbl[:], xd_out[:])

                # out_proj weight prefetch (overlaps dt + scan)
                nc.sync.dma_start(
                    outw_s[:], outw8.rearrange("(o p) e -> p o e", p=P))

                # dt proj + softplus (exp then ln)
                dtw = pA.tile([DTR, DIH], BF16, tag="dtw")
                nc.sync.dma_start(dtw[:], dtw_t[:])
                dtraw = pA.tile([DTR, L], BF16, tag="dtraw")
                nc.vector.tensor_copy(dtraw[:], xdbl[0:DTR, :])
                for m in range(DT):
                    psd = [psum.tile([P, 512], F32, tag="mm",
                                     name=f"pd{h}") for h in range(2)]
                    for h in range(2):
                        nc.tensor.matmul(
                            psd[h][:], dtw[:, m * P:(m + 1) * P],
                            dtraw[:, h * 512:(h + 1) * 512],
                            start=True, stop=True)
                        nc.scalar.activation(
                            delta[:, m, h * 512:(h + 1) * 512],
                            psd[h][:], Act.Exp, bias=dtb[:, m:m + 1])
                for m in range(DT):
                    nc.scalar.activation(delta[:, m, :], delta[:, m, :],
                                         Act.Ln, bias=1.0)

                # du = delta*u ; uD = u*D (u dies with pA)
                for d in range(DT):
                    nc.vector.tensor_tensor(du[:, d, :], delta[:, d, :],
                                            u_bf[:, d, :], Alu.mult)
                    nc.vector.tensor_scalar_mul(uD[:, d, :], u_bf[:, d, :],
                                                dpv[:, d:d + 1])

                # B/C rows to DRAM for broadcast
                bc_bf = pA.tile([2 * NST, L], F16, tag="bcs")
                nc.vector.tensor_copy(bc_bf[:], xdbl[DTR:96, :])
                nc.sync.dma_start(bc_dram[:], bc_bf[:])

            # ---------- Phase 6: selective scan ----------
            with tc.tile_pool(name="pBC", bufs=1) as pBC, \
                 tc.tile_pool(name="pC", bufs=3) as pC:
                bc_all = pBC.tile([P, 2 * NST, L], F16, tag="bc")
                for n in range(2 * NST):
                    nc.sync.dma_start(
                        bc_all[:, n, :],
                        bc_dram[n:n + 1, :].to_broadcast((P, L)))

                idx = 0
                for d in range(DT):
                    ya = [psum.tile([P, 512], F32, tag="ya",
                                    name=f"ya{h}") for h in range(2)]
                    for n in range(NST):
                        dA = pC.tile([P, L], F16, tag="dA", name="dA")
                        nc.scalar.activation(dA[:], delta[:, d, :],
                                             Act.Exp,
                                             scale=a_sb[:, d, n:n + 1])
                        dBu = pC.tile([P, L], F16, tag="dBu", name="dBu")
                        e1 = (nc.vector if idx % 10 < MULT_VEC_OF_10
                              else nc.gpsimd)
                        e1.tensor_tensor(dBu[:], du[:, d, :],
                                         bc_all[:, n, :], Alu.mult)
                        idx += 1
                        h_t = pC.tile([P, L], F16, tag="h", name="h")
                        nc.vector.tensor_tensor_scan(
                            h_t[:], dA[:], dBu[:], 0.0, Alu.mult, Alu.add)
                        e2 = (nc.vector if idx % 10 < MULT_VEC_OF_10
                              else nc.gpsimd)
                        e2.tensor_tensor(h_t[:], h_t[:],
                                         bc_all[:, NST + n, :], Alu.mult)
                        idx += 1
                        for h in range(2):
                            nc.tensor.matmul(
                                ya[h][:], iden[:],
                                h_t[:, h * 512:(h + 1) * 512],
                                start=(n == 0), stop=False)
                    # skip-connection idmm + gate
                    ygb = pC.tile([P, L], BF16, tag="ygb", name="ygb")
                    for h in range(2):
                        sl = slice(h * 512, (h + 1) * 512)
                        nc.tensor.matmul(ya[h][:], iden[:], uD[:, d, sl],
                                         start=False, stop=True)
                        t1 = scr.tile([P, 512], BF16, tag="sm", name="t1")
                        nc.scalar.copy(t1[:], ya[h][:])
                        nc.vector.tensor_tensor(ygb[:, sl], t1[:],
                                                silz[:, d, sl], Alu.mult)
                    nc.scalar.copy(yg8[:, d, :], ygb[:])

            # ---------- Phase 7: out_proj (fp8 DoubleRow) + RS ----------
            for m in range(KT):
                pso = [psum.tile([P, 512], F32, tag="mm",
                                 name=f"po{h}") for h in range(2)]
                for dp in range(DT // 2):
                    for h in range(2):
                        nc.tensor.matmul(
                            pso[h][:],
                            outw_s[:, 2 * dp:2 * dp + 2, m * P:(m + 1) * P],
                            yg8[:, 2 * dp:2 * dp + 2,
                                h * 512:(h + 1) * 512],
                            start=(dp == 0), stop=(dp == DT // 2 - 1),
                            perf_mode=PerfMode.DoubleRow)
                ypf = scr.tile([P, L], BF16, tag="big", name="ypf")
                ypr = scr.tile([P, L], BF16, tag="big", name="ypr")
                ypr_rev = ypr[:, ::-1]
                for h in range(2):
                    sl = slice(h * 512, (h + 1) * 512)
                    nc.scalar.activation(ypf[:, sl], pso[h][:],
                                         Act.Identity, scale=mf_c[:, 0:1])
                    # write the rev-masked copy time-reversed so the
                    # accumulate DMA below reads with positive stride
                    nc.scalar.activation(ypr_rev[:, sl], pso[h][:],
                                         Act.Identity, scale=mr_c[:, 0:1])
                dstf = rs_in[:, 0, m * P:(m + 1) * P, :] \
                    .rearrange("q p c -> p q c")
                nc.sync.dma_start(
                    dstf, ypf[:].rearrange("p (q c) -> p q c", c=LQ))
                dstr = rs_in[:, 1, m * P:(m + 1) * P, :] \
                    .rearrange("q p c -> p q c")
                nc.sync.dma_start(
                    dstr, ypr[:].rearrange("p (q c) -> p q c", c=LQ))

            nc.gpsimd.collective_compute(
                "ReduceScatter", Alu.add, QUADS,
                ins=[rs_in.opt()], outs=[rs_out.opt()])

        # ---------- Phase 8: residual, x2 exchange, LN2 ----------
        with tc.tile_pool(name="pE", bufs=1) as pE:
            # FFN half-weights (DMAs depend only on external inputs, so
            # they overlap everything before the FFN)
            w1s = pE.tile([P, KT, DFH], BF16, tag="w1s")
            nc.sync.dma_start(w1s[:],
                              w1h_t.rearrange("(o p) e -> p o e", p=P))
            w2s = pE.tile([P, DFH // P, DM], BF16, tag="w2s")
            nc.sync.dma_start(w2s[:],
                              w2h_t.rearrange("(o p) e -> p o e", p=P))

            x2 = pE.tile([P, KT, LQ], F32, tag="x2")
            nc.sync.dma_start(
                x2[:], x_res.rearrange("(o p) t -> p o t", p=P))
            rsl = pE.tile([P, 2, KT, LQ], BF16, tag="rsl")
            for sdx in range(2):
                nc.sync.dma_start(
                    rsl[:, sdx, :, :],
                    rs_out[sdx].rearrange("(o p) t -> p o t", p=P))
            nc.vector.tensor_tensor(
                rsl[:, 0, :, :].rearrange("p o t -> p (o t)"),
                rsl[:, 0, :, :].rearrange("p o t -> p (o t)"),
                rsl[:, 1, :, :].rearrange("p o t -> p (o t)"), Alu.add)
            nc.vector.tensor_tensor(
                x2[:].rearrange("p o t -> p (o t)"),
                x2[:].rearrange("p o t -> p (o t)"),
                rsl[:, 0, :, :].rearrange("p o t -> p (o t)"), Alu.add)

            # share the quarter with the pair (bf16) -> L-half x2h
            x2qb = pE.tile([P, KT, LQ], BF16, tag="x2qb")
            for kt in range(KT):
                nc.scalar.copy(x2qb[:, kt, :], x2[:, kt, :])
            nc.sync.dma_start(
                agx_in.rearrange("(o p) t -> p o t", p=P), x2qb[:])
            nc.gpsimd.collective_compute(
                "AllGather", Alu.bypass, PAIRS,
                ins=[agx_in.opt()], outs=[agx_out.opt()])
            x2h = pE.tile([P, KT, 2, LQ], BF16, tag="x2h")
            for sdx in range(2):
                nc.sync.dma_start(
                    x2h[:, :, sdx, :],
                    agx_out[sdx].rearrange("(o p) t -> p o t", p=P))

            # LN2 over the L-half
            ps_m2 = psum.tile([1, LH], F32, tag="st", name="m2q")
            ps_s2 = psum.tile([1, LH], F32, tag="st2", name="s2q")
            x2hf = x2h[:].rearrange("p o s t -> p o (s t)")
            for kt in range(KT):
                sq = scr.tile([P, LH], BF16, tag="sm", name="sq2")
                nc.scalar.square(sq[:], x2hf[:, kt, :])
                nc.tensor.matmul(ps_m2[:], ones_bf[:], x2hf[:, kt, :],
                                 start=(kt == 0), stop=(kt == KT - 1))
                nc.tensor.matmul(ps_s2[:], ones_bf[:], sq[:],
                                 start=(kt == 0), stop=(kt == KT - 1))
            mean2 = statp.tile([1, LH], F32, tag="stat", name="mean2")
            var2 = statp.tile([1, LH], F32, tag="stat", name="var2")
            m22 = statp.tile([1, LH], F32, tag="stat", name="m22")
            nc.vector.tensor_scalar_mul(mean2[:], ps_m2[:], 1.0 / DM)
            nc.vector.tensor_scalar_mul(var2[:], ps_s2[:], 1.0 / DM)
            nc.vector.tensor_tensor(m22[:], mean2[:], mean2[:], Alu.mult)
            nc.vector.tensor_tensor(var2[:], var2[:], m22[:], Alu.subtract)
            nc.vector.tensor_scalar_add(var2[:], var2[:], EPS)
            nc.scalar.sqrt(var2[:], var2[:])
            nc.vector.reciprocal(m22[:], var2[:])
            mean2_bc = pE.tile([P, LH], F32, tag="m2bc")
            rstd2_bc = pE.tile([P, LH], F32, tag="r2bc")
            nc.gpsimd.partition_broadcast(mean2_bc[:], mean2[:])
            nc.gpsimd.partition_broadcast(rstd2_bc[:], m22[:])
            x2n = pE.tile([P, KT, LH], BF16, tag="x2n")
            for kt in range(KT):
                t1 = scr.tile([P, LH], F32, tag="sm", name="t2")
                nc.vector.tensor_tensor(t1[:], x2hf[:, kt, :], mean2_bc[:],
                                        Alu.subtract)
                nc.vector.tensor_tensor(x2n[:, kt, :], t1[:], rstd2_bc[:],
                                        Alu.mult)

            # ---------- Phase 9: FFN (d_ff half x L half) ----------
            h1 = pE.tile([P, DFH // P, LH], BF16, tag="h1")
            for mq in range(DFH // P):
                psf = psum.tile([P, LH], F32, tag="ya", name="pf")
                for k in range(KT):
                    nc.tensor.matmul(
                        psf[:], w1s[:, k, mq * P:(mq + 1) * P],
                        x2n[:, k, :], start=(k == 0), stop=(k == KT - 1))
                nc.scalar.activation(h1[:, mq, :], psf[:], Act.Gelu,
                                     bias=b1s[:, mq:mq + 1])
            ffp = pE.tile([P, KT, LH], BF16, tag="ffp")
            for m in range(KT):
                psg = psum.tile([P, LH], F32, tag="ya", name="pg")
                for k2 in range(DFH // P):
                    nc.tensor.matmul(
                        psg[:], w2s[:, k2, m * P:(m + 1) * P],
                        h1[:, k2, :],
                        start=(k2 == 0), stop=(k2 == DFH // P - 1))
                nc.scalar.copy(ffp[:, m, :], psg[:])
            for sdx in range(2):
                nc.sync.dma_start(
                    rsf_in[sdx].rearrange("(o p) t -> p o t", p=P),
                    ffp[:, :, sdx * LQ:(sdx + 1) * LQ])
            nc.gpsimd.collective_compute(
                "ReduceScatter", Alu.add, PAIRS,
                ins=[rsf_in.opt()], outs=[rsf_out.opt()])

            # final: out_q = x2_q + ffn_q + b2 -> AllGather quad
            ffq = pE.tile([P, KT, LQ], BF16, tag="ffq")
            nc.sync.dma_start(
                ffq[:], rsf_out.rearrange("(o p) t -> p o t", p=P))
            oq = pE.tile([P, KT, LQ], BF16, tag="oq")
            for kt in range(KT):
                nc.vector.tensor_tensor(oq[:, kt, :], x2[:, kt, :],
                                        ffq[:, kt, :], Alu.add)
                nc.vector.tensor_scalar(oq[:, kt, :], oq[:, kt, :], 1.0,
                                        b2s[:, kt:kt + 1], Alu.mult,
                                        Alu.add)
            nc.sync.dma_start(
                ag_in.rearrange("(o p) t -> p o t", p=P), oq[:])
            nc.gpsimd.collective_compute(
                "AllGather", Alu.bypass, QUADS,
                ins=[ag_in.opt()], outs=[ag_out.opt()])
            nc.sync.dma_start(outT[:], ag_out[:])

    nc.compile()
    return nc


_NC_CACHE = None


def _get_nc():
    global _NC_CACHE
    if _NC_CACHE is None:
        _NC_CACHE = build_program()
    return _NC_CACHE


def _q8(x, scale):
    v = np.clip(np.asarray(x, np.float64) * scale, -230.0, 230.0)
    return np.asarray(v).astype(_F8)


def _prep_core(inputs, b, dir_, half):
    hs = slice(half * DIH, (half + 1) * DIH)
    p = "f_" if dir_ == 0 else "b_"
    rank = 2 * dir_ + half
    f32 = np.float32
    m = {}
    xT = np.ascontiguousarray(inputs["x"][b].T.astype(f32))
    xl = xT if dir_ == 0 else np.ascontiguousarray(xT[:, ::-1])
    m["xT_ln"] = xl.astype(_BF)
    m["x_res"] = np.ascontiguousarray(xT[:, rank * LQ:(rank + 1) * LQ])

    W = inputs[p + "in_proj_w"]
    win = np.concatenate(
        [W[hs], W[2 * DIH + half * DIH:2 * DIH + (half + 1) * DIH]],
        axis=0)
    g1 = inputs["norm_g"].astype(np.float64)
    b1n = inputs["norm_b"].astype(np.float64)
    win_eff = win.astype(np.float64) * g1[None, :]
    m["win8"] = np.ascontiguousarray(_q8(win_eff.T, S_W))
    m["win_b"] = (win.astype(np.float64) @ b1n).astype(f32)
    m["sinv"] = np.full((P,), 1.0 / (S_W * S_X), f32)
    m["conv_w"] = np.ascontiguousarray(
        inputs[p + "conv_w"][hs, 0, :]).astype(f32)
    m["conv_b"] = inputs[p + "conv_b"][hs].astype(f32)
    m["a_mat"] = (-np.exp(inputs[p + "A_log"][hs])).astype(f32)
    m["xpw_t"] = np.ascontiguousarray(
        inputs[p + "x_proj_w"][:, hs].T).astype(_BF)
    m["dtw_t"] = np.ascontiguousarray(
        inputs[p + "dt_proj_w"][hs].T).astype(_BF)
    m["dt_b"] = inputs[p + "dt_proj_b"][hs].astype(f32)
    m["d_par"] = inputs[p + "D"][hs].astype(f32)
    m["outw8"] = np.ascontiguousarray(
        _q8(0.5 * inputs[p + "out_proj_w"][:, hs].T, S_O))
    mval = 1.0 / (S_O * S_YG)
    m["mf_in"] = np.full((P,), mval if dir_ == 0 else 0.0, f32)
    m["mr_in"] = np.full((P,), mval if dir_ == 1 else 0.0, f32)
    m["iden_in"] = (np.eye(P) * S_YG).astype(np.float16)

    # FFN: d_ff half by rank parity; the pair shares the L-half
    fh = rank % 2
    fsl = slice(fh * DFH, (fh + 1) * DFH)
    g2 = inputs["ffn_g"].astype(np.float64)
    b2n = inputs["ffn_b"].astype(np.float64)
    w1 = inputs["w1"].astype(np.float64)[fsl]
    m["w1h_t"] = np.ascontiguousarray((w1 * g2[None, :]).T).astype(_BF)
    m["b1_h"] = (inputs["b1"][fsl] + w1 @ b2n).astype(f32)
    m["w2h_t"] = np.ascontiguousarray(inputs["w2"][:, fsl].T).astype(_BF)
    m["b2_e"] = inputs["b2"].astype(f32)
    return m


def make_in_maps(inputs):
    inputs = {k: np.asarray(v) for k, v in inputs.items()}
    maps = []
    for c in range(8):
        b, dir_, half = c // 4, (c // 2) % 2, c % 2
        maps.append(_prep_core(inputs, b, dir_, half))
    return maps


def kernel(**inputs):
    from concourse.bass_utils import run_bass_kernel_spmd
    nc = _get_nc()
    in_maps = make_in_maps(inputs)
    res = run_bass_kernel_spmd(nc, in_maps, core_ids=list(range(8)))
    outs = []
    for c in (0, 4):
        o = res.results[c]["outT"]  # [4, DM, LQ]
        full = np.concatenate([o[q] for q in range(4)], axis=1)
        outs.append(full.T)
    return np.stack(outs).astype(np.float32)


# revision 3
# speedup vs baseline: 1.0104x; 1.0104x over previous
"""BiMamba block Trainium2 kernel v2 — 8 NeuronCores.

Sharding: core c = 4*b + 2*dir + half (b batch, dir direction, half of
d_inner).  rank = c % 4 = 2*dir + half also indexes the forward-time
L-quarter this core owns after the y-combine ReduceScatter.

- in_proj / out_proj in fp8 (e4m3) with DoubleRow matmuls.
- Scan n-accumulation via identity-matmul into PSUM (iden carries S_YG,
  and an extra u*D idmm term folds the skip connection in); scan ops on
  Vector (only engine with the scan), multiplies split Vector/GpSimd.
- Direction combine via ONE quad ReduceScatter over forward-time
  quarters: each core evacuates its out_proj partial twice (masked by
  per-core fwd/rev inputs); the reversed copy is DMA-accumulated with a
  negative-stride access pattern, so dir=1 contributions land
  time-flipped with an SPMD-uniform program.
- FFN: within each L-half pair, cores split d_ff in half and share the
  L-half (AllGather-pair of x2 quarters), partials combined with a
  ReduceScatter-pair; final quad AllGather assembles outT.
"""

import numpy as np
import ml_dtypes

import concourse.bass as bass
import concourse.bacc as bacc
import concourse.mybir as mybir
import concourse.tile as tile
from contextlib import ExitStack

F32 = mybir.dt.float32
F16 = mybir.dt.float16
BF16 = mybir.dt.bfloat16
FP8 = mybir.dt.float8e4
Alu = mybir.AluOpType
Act = mybir.ActivationFunctionType
PerfMode = mybir.MatmulPerfMode

P = 128
DM = 1024
L = 1024
DIH = 1024
NST = 16
DTR = 64
KT = DM // P        # 8
DT = DIH // P       # 8
LQ = L // 4         # 256
LH = L // 2         # 512
DFH = 2048          # d_ff half
EPS = 1e-5

S_X = 32.0          # xnorm fp8 scale
S_W = 1024.0        # in_proj weight fp8 scale
S_YG = 128.0        # gated-y fp8 scale
S_O = 2048.0        # out_proj weight fp8 scale

_BF = ml_dtypes.bfloat16
_F8 = ml_dtypes.float8_e4m3

PAIRS = [[0, 1], [2, 3], [4, 5], [6, 7]]
QUADS = [[0, 1, 2, 3], [4, 5, 6, 7]]

# Elementwise multiplies in the scan: op j goes to Vector when
# (j % 10) < MULT_VEC_OF_10, else GpSimd.  GpSimd shares its SBUF port
# with the DVE, so offloading there slows the scan ops ~1.6x — keep
# everything on Vector.
MULT_VEC_OF_10 = 10


def build_program():
    nc = bacc.Bacc("TRN2", target_bir_lowering=False, debug=False,
                   num_devices=8)

    # ---- I/O ----
    xT_ln = nc.dram_tensor("xT_ln", [DM, L], BF16, kind="ExternalInput")
    x_res = nc.dram_tensor("x_res", [DM, LQ], F32, kind="ExternalInput")
    win8 = nc.dram_tensor("win8", [DM, 2 * DIH], FP8, kind="ExternalInput")
    win_b = nc.dram_tensor("win_b", [2 * DIH], F32, kind="ExternalInput")
    sinv = nc.dram_tensor("sinv", [P], F32, kind="ExternalInput")
    conv_w = nc.dram_tensor("conv_w", [DIH, 4], F32, kind="ExternalInput")
    conv_b = nc.dram_tensor("conv_b", [DIH], F32, kind="ExternalInput")
    a_mat = nc.dram_tensor("a_mat", [DIH, NST], F32, kind="ExternalInput")
    xpw_t = nc.dram_tensor("xpw_t", [DIH, 96], BF16, kind="ExternalInput")
    dtw_t = nc.dram_tensor("dtw_t", [DTR, DIH], BF16, kind="ExternalInput")
    dt_b = nc.dram_tensor("dt_b", [DIH], F32, kind="ExternalInput")
    d_par = nc.dram_tensor("d_par", [DIH], F32, kind="ExternalInput")
    outw8 = nc.dram_tensor("outw8", [DIH, DM], FP8, kind="ExternalInput")
    mf_in = nc.dram_tensor("mf_in", [P], F32, kind="ExternalInput")
    mr_in = nc.dram_tensor("mr_in", [P], F32, kind="ExternalInput")
    iden_in = nc.dram_tensor("iden_in", [P, P], F16, kind="ExternalInput")
    w1h_t = nc.dram_tensor("w1h_t", [DM, DFH], BF16, kind="ExternalInput")
    b1_h = nc.dram_tensor("b1_h", [DFH], F32, kind="ExternalInput")
    w2h_t = nc.dram_tensor("w2h_t", [DFH, DM], BF16, kind="ExternalInput")
    b2_e = nc.dram_tensor("b2_e", [DM], F32, kind="ExternalInput")
    outT = nc.dram_tensor("outT", [4, DM, LQ], BF16, kind="ExternalOutput")

    def vec_pt(dram_vec, pool, dt_, tag):
        t = pool.tile([P, dram_vec.shape[0] // P], dt_, tag=tag, name=tag)
        nc.sync.dma_start(t[:], dram_vec.rearrange("(o p) -> p o", p=P))
        return t

    def col_pt(dram_vec, pool, tag):
        t = pool.tile([P, 1], F32, tag=tag, name=tag)
        nc.sync.dma_start(t[:], dram_vec.rearrange("(p o) -> p o", o=1))
        return t

    with tile.TileContext(nc) as tc, ExitStack() as es:
        pc = es.enter_context(tc.tile_pool(name="const", bufs=1))
        psum = es.enter_context(tc.tile_pool(name="psum", bufs=2,
                                             space="PSUM"))
        scr = es.enter_context(tc.tile_pool(name="scr", bufs=3))
        statp = es.enter_context(tc.tile_pool(name="statp", bufs=3))
        dram = es.enter_context(tc.tile_pool(name="dram", bufs=1,
                                             space="DRAM"))

        # constants
        ones_bf = pc.tile([P, 1], BF16, tag="onesb")
        nc.vector.memset(ones_bf[:], 1.0)
        wbv = vec_pt(win_b, pc, F32, "wbv")
        sinv_c = col_pt(sinv, pc, "sinv")
        cw = pc.tile([P, DT, 4], F32, tag="cw")
        nc.sync.dma_start(cw[:], conv_w.rearrange("(o p) k -> p o k", p=P))
        cb = vec_pt(conv_b, pc, F32, "cb")
        a_sb = pc.tile([P, DT, NST], F32, tag="a")
        nc.sync.dma_start(a_sb[:], a_mat.rearrange("(o p) n -> p o n", p=P))
        dtb = vec_pt(dt_b, pc, F32, "dtb")
        dpv = vec_pt(d_par, pc, F32, "dpv")
        mf_c = col_pt(mf_in, pc, "mfc")
        mr_c = col_pt(mr_in, pc, "mrc")
        iden = pc.tile([P, P], F16, tag="iden")
        nc.sync.dma_start(iden[:], iden_in[:])
        b1s = vec_pt(b1_h, pc, F32, "b1s")
        b2s = vec_pt(b2_e, pc, F32, "b2s")

        rs_in = dram.tile([4, DM, LQ], BF16)
        rs_out = dram.tile([DM, LQ], BF16)
        agx_in = dram.tile([DM, LQ], BF16)
        agx_out = dram.tile([2, DM, LQ], BF16)
        rsf_in = dram.tile([2, DM, LQ], BF16)
        rsf_out = dram.tile([DM, LQ], BF16)
        ag_in = dram.tile([DM, LQ], BF16)
        ag_out = dram.tile([4, DM, LQ], BF16)
        xd_in = dram.tile([96, L], F32)
        xd_out = dram.tile([96, L], F32)
        bc_dram = dram.tile([2 * NST, L], F16)

        with tc.tile_pool(name="pS", bufs=1) as pS:
            # persistent through scan + out_proj
            delta = pS.tile([P, DT, L], F16, tag="delta")
            du = pS.tile([P, DT, L], F16, tag="du")
            uD = pS.tile([P, DT, L], F16, tag="uD")
            silz = pS.tile([P, DT, L], BF16, tag="silz")
            yg8 = pS.tile([P, DT, L], FP8, tag="yg8")
            outw_s = pS.tile([P, DT, DM], FP8, tag="outw")
            xdbl = pS.tile([96, L], F32, tag="xdbl")

            # ---------- Phases 0-5 (pA scope) ----------
            with tc.tile_pool(name="pA", bufs=1) as pA, \
                 tc.tile_pool(name="pAx", bufs=3) as pAx:
                win_s = pA.tile([P, KT, 2 * DIH], FP8, tag="win")
                nc.sync.dma_start(
                    win_s[:], win8.rearrange("(o p) e -> p o e", p=P))
                u_bf = pA.tile([P, DT, L], BF16, tag="u")

                xlnv = xT_ln.rearrange("(o p) t -> p o t", p=P)

                def _load_x(kt):
                    xk = pAx.tile([P, L], BF16, tag="xk", name="xk")
                    nc.sync.dma_start(xk[:], xlnv[:, kt, :])
                    return xk[:]

                # LN1 stats
                ps_m = [psum.tile([1, 512], F32, tag="st", name=f"m{h}")
                        for h in range(2)]
                ps_s = [psum.tile([1, 512], F32, tag="st2", name=f"s{h}")
                        for h in range(2)]
                for kt in range(KT):
                    xk = _load_x(kt)
                    sq = scr.tile([P, L], BF16, tag="big", name="sq")
                    nc.scalar.square(sq[:], xk)
                    for h in range(2):
                        sl = slice(h * 512, (h + 1) * 512)
                        nc.tensor.matmul(ps_m[h][:], ones_bf[:], xk[:, sl],
                                         start=(kt == 0),
                                         stop=(kt == KT - 1))
                        nc.tensor.matmul(ps_s[h][:], ones_bf[:], sq[:, sl],
                                         start=(kt == 0),
                                         stop=(kt == KT - 1))
                mean1 = statp.tile([1, L], F32, tag="stat", name="mean1")
                var1 = statp.tile([1, L], F32, tag="stat", name="var1")
                m2 = statp.tile([1, L], F32, tag="stat", name="m2")
                for h in range(2):
                    sl = slice(h * 512, (h + 1) * 512)
                    nc.vector.tensor_scalar_mul(mean1[:, sl], ps_m[h][:],
                                                1.0 / DM)
                    nc.vector.tensor_scalar_mul(var1[:, sl], ps_s[h][:],
                                                1.0 / DM)
                nc.vector.tensor_tensor(m2[:], mean1[:], mean1[:], Alu.mult)
                nc.vector.tensor_tensor(var1[:], var1[:], m2[:],
                                        Alu.subtract)
                nc.vector.tensor_scalar_add(var1[:], var1[:], EPS)
                nc.scalar.sqrt(var1[:], var1[:])
                nc.vector.reciprocal(m2[:], var1[:])
                nc.vector.tensor_scalar_mul(m2[:], m2[:], S_X)
                mean_bc = pA.tile([P, L], F32, tag="meanbc")
                rstd_bc = pA.tile([P, L], F32, tag="rstdbc")
                nc.gpsimd.partition_broadcast(mean_bc[:], mean1[:])
                nc.gpsimd.partition_broadcast(rstd_bc[:], m2[:])

                # LN1 apply -> fp8 (scaled by S_X via rstd)
                xn8 = pA.tile([P, KT, L], FP8, tag="xn8")
                for kt in range(KT):
                    xk = _load_x(kt)
                    t1 = scr.tile([P, L], F32, tag="big", name="t1")
                    nc.vector.tensor_tensor(t1[:], xk, mean_bc[:],
                                            Alu.subtract)
                    t2 = scr.tile([P, L], BF16, tag="big", name="t2")
                    nc.vector.tensor_tensor(t2[:], t1[:], rstd_bc[:],
                                            Alu.mult)
                    nc.scalar.copy(xn8[:, kt, :], t2[:])

                # in_proj (fp8 DoubleRow)
                xiT = pA.tile([P, DT, L + 3], BF16, tag="xi")
                nc.vector.memset(xiT[:, :, 0:3], 0.0)
                for m in range(2 * DT):
                    pss = [psum.tile([P, 512], F32, tag="mm",
                                     name=f"pss{h}") for h in range(2)]
                    for kp in range(KT // 2):
                        for h in range(2):
                            nc.tensor.matmul(
                                pss[h][:],
                                win_s[:, 2 * kp:2 * kp + 2,
                                      m * P:(m + 1) * P],
                                xn8[:, 2 * kp:2 * kp + 2,
                                    h * 512:(h + 1) * 512],
                                start=(kp == 0), stop=(kp == KT // 2 - 1),
                                perf_mode=PerfMode.DoubleRow)
                    for h in range(2):
                        if m < DT:
                            nc.scalar.activation(
                                xiT[:, m, 3 + h * 512:3 + (h + 1) * 512],
                                pss[h][:], Act.Identity,
                                bias=wbv[:, m:m + 1], scale=sinv_c[:, 0:1])
                        else:
                            nc.scalar.activation(
                                silz[:, m - DT, h * 512:(h + 1) * 512],
                                pss[h][:], Act.Silu,
                                bias=wbv[:, m:m + 1], scale=sinv_c[:, 0:1])

                # conv + silu
                for d in range(DT):
                    acc = scr.tile([P, L], BF16, tag="big", name="acc")
                    nc.vector.tensor_scalar(
                        acc[:], xiT[:, d, 0:L], cw[:, d, 0:1],
                        cb[:, d:d + 1], Alu.mult, Alu.add)
                    for k in (1, 2, 3):
                        nc.vector.scalar_tensor_tensor(
                            acc[:], xiT[:, d, k:k + L], cw[:, d, k:k + 1],
                            acc[:], Alu.mult, Alu.add)
                    nc.scalar.activation(u_bf[:, d, :], acc[:], Act.Silu)

                # x_proj + pair AllReduce
                xpw = pA.tile([P, DT, 96], BF16, tag="xpw")
                nc.sync.dma_start(
                    xpw[:], xpw_t.rearrange("(o p) r -> p o r", p=P))
                psx = [psum.tile([96, 512], F32, tag="mm", name=f"px{h}")
                       for h in range(2)]
                for d in range(DT):
                    for h in range(2):
                        nc.tensor.matmul(
                            psx[h][:], xpw[:, d, :],
                            u_bf[:, d, h * 512:(h + 1) * 512],
                            start=(d == 0), stop=(d == DT - 1))
                xdp = scr.tile([P, L], F32, tag="big", name="xdp")
                for h in range(2):
                    nc.scalar.copy(xdp[0:96, h * 512:(h + 1) * 512],
                                   psx[h][:])
                nc.sync.dma_start(xd_in[:], xdp[0:96, :])
                nc.gpsimd.collective_compute(
                    "AllReduce", Alu.add, PAIRS,
                    ins=[xd_in.opt()], outs=[xd_out.opt()])
                nc.sync.dma_start(xdbl[:], xd_out[:])

                # out_proj weight prefetch (overlaps dt + scan)
                nc.sync.dma_start(
                    outw_s[:], outw8.rearrange("(o p) e -> p o e", p=P))

                # dt proj + softplus (exp then ln)
                dtw = pA.tile([DTR, DIH], BF16, tag="dtw")
                nc.sync.dma_start(dtw[:], dtw_t[:])
                dtraw = pA.tile([DTR, L], BF16, tag="dtraw")
                nc.vector.tensor_copy(dtraw[:], xdbl[0:DTR, :])
                for m in range(DT):
                    psd = [psum.tile([P, 512], F32, tag="mm",
                                     name=f"pd{h}") for h in range(2)]
                    for h in range(2):
                        nc.tensor.matmul(
                            psd[h][:], dtw[:, m * P:(m + 1) * P],
                            dtraw[:, h * 512:(h + 1) * 512],
                            start=True, stop=True)
                        nc.scalar.activation(
                            delta[:, m, h * 512:(h + 1) * 512],
                            psd[h][:], Act.Exp, bias=dtb[:, m:m + 1])
                for m in range(DT):
                    nc.scalar.activation(delta[:, m, :], delta[:, m, :],
                                         Act.Ln, bias=1.0)

                # du = delta*u ; uD = u*D (u dies with pA)
                for d in range(DT):
                    nc.vector.tensor_tensor(du[:, d, :], delta[:, d, :],
                                            u_bf[:, d, :], Alu.mult)
                    nc.vector.tensor_scalar_mul(uD[:, d, :], u_bf[:, d, :],
                                                dpv[:, d:d + 1])

                # B/C rows to DRAM for broadcast
                bc_bf = pA.tile([2 * NST, L], F16, tag="bcs")
                nc.vector.tensor_copy(bc_bf[:], xdbl[DTR:96, :])
                nc.sync.dma_start(bc_dram[:], bc_bf[:])

            # ---------- Phase 6: selective scan ----------
            with tc.tile_pool(name="pBC", bufs=1) as pBC, \
                 tc.tile_pool(name="pC", bufs=3) as pC:
                bc_all = pBC.tile([P, 2 * NST, L], F16, tag="bc")
                for n in range(2 * NST):
                    nc.sync.dma_start(
                        bc_all[:, n, :],
                        bc_dram[n:n + 1, :].to_broadcast((P, L)))

                idx = 0
                for d in range(DT):
                    ya = [psum.tile([P, 512], F32, tag="ya",
                                    name=f"ya{h}") for h in range(2)]
                    for n in range(NST):
                        dA = pC.tile([P, L], F16, tag="dA", name="dA")
                        nc.scalar.activation(dA[:], delta[:, d, :],
                                             Act.Exp,
                                             scale=a_sb[:, d, n:n + 1])
                        dBu = pC.tile([P, L], F16, tag="dBu", name="dBu")
                        e1 = (nc.vector if idx % 10 < MULT_VEC_OF_10
                              else nc.gpsimd)
                        e1.tensor_tensor(dBu[:], du[:, d, :],
                                         bc_all[:, n, :], Alu.mult)
                        idx += 1
                        h_t = pC.tile([P, L], F16, tag="h", name="h")
                        nc.vector.tensor_tensor_scan(
                            h_t[:], dA[:], dBu[:], 0.0, Alu.mult, Alu.add)
                        e2 = (nc.vector if idx % 10 < MULT_VEC_OF_10
                              else nc.gpsimd)
                        e2.tensor_tensor(h_t[:], h_t[:],
                                         bc_all[:, NST + n, :], Alu.mult)
                        idx += 1
                        for h in range(2):
                            nc.tensor.matmul(
                                ya[h][:], iden[:],
                                h_t[:, h * 512:(h + 1) * 512],
                                start=(n == 0), stop=False)
                    # skip-connection idmm + gate
                    ygb = pC.tile([P, L], BF16, tag="ygb", name="ygb")
                    for h in range(2):
                        sl = slice(h * 512, (h + 1) * 512)
                        nc.tensor.matmul(ya[h][:], iden[:], uD[:, d, sl],
                                         start=False, stop=True)
                        t1 = scr.tile([P, 512], BF16, tag="sm", name="t1")
                        nc.scalar.copy(t1[:], ya[h][:])
                        nc.vector.tensor_tensor(ygb[:, sl], t1[:],
                                                silz[:, d, sl], Alu.mult)
                    nc.scalar.copy(yg8[:, d, :], ygb[:])

            # ---------- Phase 7: out_proj (fp8 DoubleRow) + RS ----------
            for m in range(KT):
                pso = [psum.tile([P, 512], F32, tag="mm",
                                 name=f"po{h}") for h in range(2)]
                for dp in range(DT // 2):
                    for h in range(2):
                        nc.tensor.matmul(
                            pso[h][:],
                            outw_s[:, 2 * dp:2 * dp + 2, m * P:(m + 1) * P],
                            yg8[:, 2 * dp:2 * dp + 2,
                                h * 512:(h + 1) * 512],
                            start=(dp == 0), stop=(dp == DT // 2 - 1),
                            perf_mode=PerfMode.DoubleRow)
                ypf = scr.tile([P, L], BF16, tag="big", name="ypf")
                ypr = scr.tile([P, L], BF16, tag="big", name="ypr")
                ypr_rev = ypr[:, ::-1]
                for h in range(2):
                    sl = slice(h * 512, (h + 1) * 512)
                    nc.scalar.activation(ypf[:, sl], pso[h][:],
                                         Act.Identity, scale=mf_c[:, 0:1])
                    # write the rev-masked copy time-reversed so the
                    # accumulate DMA below reads with positive stride
                    nc.scalar.activation(ypr_rev[:, sl], pso[h][:],
                                         Act.Identity, scale=mr_c[:, 0:1])
                dst = rs_in[:, m * P:(m + 1) * P, :] \
                    .rearrange("q p c -> p q c")
                nc.sync.dma_start(
                    dst, ypf[:].rearrange("p (q c) -> p q c", c=LQ))
                nc.gpsimd.dma_start(
                    dst, ypr[:].rearrange("p (q c) -> p q c", c=LQ),
                    accum_op=Alu.add)

            nc.gpsimd.collective_compute(
                "ReduceScatter", Alu.add, QUADS,
                ins=[rs_in.opt()], outs=[rs_out.opt()])

        # ---------- Phase 8: residual, x2 exchange, LN2 ----------
        with tc.tile_pool(name="pE", bufs=1) as pE:
            # FFN half-weights (DMAs depend only on external inputs, so
            # they overlap everything before the FFN)
            w1s = pE.tile([P, KT, DFH], BF16, tag="w1s")
            nc.sync.dma_start(w1s[:],
                              w1h_t.rearrange("(o p) e -> p o e", p=P))
            w2s = pE.tile([P, DFH // P, DM], BF16, tag="w2s")
            nc.sync.dma_start(w2s[:],
                              w2h_t.rearrange("(o p) e -> p o e", p=P))

            x2 = pE.tile([P, KT, LQ], F32, tag="x2")
            nc.sync.dma_start(
                x2[:], x_res.rearrange("(o p) t -> p o t", p=P))
            rsl = pE.tile([P, KT, LQ], BF16, tag="rsl")
            nc.sync.dma_start(
                rsl[:], rs_out.rearrange("(o p) t -> p o t", p=P))
            nc.vector.tensor_tensor(
                x2[:].rearrange("p o t -> p (o t)"),
                x2[:].rearrange("p o t -> p (o t)"),
                rsl[:].rearrange("p o t -> p (o t)"), Alu.add)

            # share the quarter with the pair (bf16) -> L-half x2h
            x2qb = pE.tile([P, KT, LQ], BF16, tag="x2qb")
            for kt in range(KT):
                nc.scalar.copy(x2qb[:, kt, :], x2[:, kt, :])
            nc.sync.dma_start(
                agx_in.rearrange("(o p) t -> p o t", p=P), x2qb[:])
            nc.gpsimd.collective_compute(
                "AllGather", Alu.bypass, PAIRS,
                ins=[agx_in.opt()], outs=[agx_out.opt()])
            x2h = pE.tile([P, KT, 2, LQ], BF16, tag="x2h")
            for sdx in range(2):
                nc.sync.dma_start(
                    x2h[:, :, sdx, :],
                    agx_out[sdx].rearrange("(o p) t -> p o t", p=P))

            # LN2 over the L-half
            ps_m2 = psum.tile([1, LH], F32, tag="st", name="m2q")
            ps_s2 = psum.tile([1, LH], F32, tag="st2", name="s2q")
            x2hf = x2h[:].rearrange("p o s t -> p o (s t)")
            for kt in range(KT):
                sq = scr.tile([P, LH], BF16, tag="sm", name="sq2")
                nc.scalar.square(sq[:], x2hf[:, kt, :])
                nc.tensor.matmul(ps_m2[:], ones_bf[:], x2hf[:, kt, :],
                                 start=(kt == 0), stop=(kt == KT - 1))
                nc.tensor.matmul(ps_s2[:], ones_bf[:], sq[:],
                                 start=(kt == 0), stop=(kt == KT - 1))
            mean2 = statp.tile([1, LH], F32, tag="stat", name="mean2")
            var2 = statp.tile([1, LH], F32, tag="stat", name="var2")
            m22 = statp.tile([1, LH], F32, tag="stat", name="m22")
            nc.vector.tensor_scalar_mul(mean2[:], ps_m2[:], 1.0 / DM)
            nc.vector.tensor_scalar_mul(var2[:], ps_s2[:], 1.0 / DM)
            nc.vector.tensor_tensor(m22[:], mean2[:], mean2[:], Alu.mult)
            nc.vector.tensor_tensor(var2[:], var2[:], m22[:], Alu.subtract)
            nc.vector.tensor_scalar_add(var2[:], var2[:], EPS)
            nc.scalar.sqrt(var2[:], var2[:])
            nc.vector.reciprocal(m22[:], var2[:])
            mean2_bc = pE.tile([P, LH], F32, tag="m2bc")
            rstd2_bc = pE.tile([P, LH], F32, tag="r2bc")
            nc.gpsimd.partition_broadcast(mean2_bc[:], mean2[:])
            nc.gpsimd.partition_broadcast(rstd2_bc[:], m22[:])
            x2n = pE.tile([P, KT, LH], BF16, tag="x2n")
            for kt in range(KT):
                t1 = scr.tile([P, LH], F32, tag="sm", name="t2")
                nc.vector.tensor_tensor(t1[:], x2hf[:, kt, :], mean2_bc[:],
                                        Alu.subtract)
                nc.vector.tensor_tensor(x2n[:, kt, :], t1[:], rstd2_bc[:],
                                        Alu.mult)

            # ---------- Phase 9: FFN (d_ff half x L half) ----------
            h1 = pE.tile([P, DFH // P, LH], BF16, tag="h1")
            for mq in range(DFH // P):
                psf = psum.tile([P, LH], F32, tag="ya", name="pf")
                for k in range(KT):
                    nc.tensor.matmul(
                        psf[:], w1s[:, k, mq * P:(mq + 1) * P],
                        x2n[:, k, :], start=(k == 0), stop=(k == KT - 1))
                nc.scalar.activation(h1[:, mq, :], psf[:], Act.Gelu,
                                     bias=b1s[:, mq:mq + 1])
            ffp = pE.tile([P, KT, LH], BF16, tag="ffp")
            for m in range(KT):
                psg = psum.tile([P, LH], F32, tag="ya", name="pg")
                for k2 in range(DFH // P):
                    nc.tensor.matmul(
                        psg[:], w2s[:, k2, m * P:(m + 1) * P],
                        h1[:, k2, :],
                        start=(k2 == 0), stop=(k2 == DFH // P - 1))
                nc.scalar.copy(ffp[:, m, :], psg[:])
            for sdx in range(2):
                nc.sync.dma_start(
                    rsf_in[sdx].rearrange("(o p) t -> p o t", p=P),
                    ffp[:, :, sdx * LQ:(sdx + 1) * LQ])
            nc.gpsimd.collective_compute(
                "ReduceScatter", Alu.add, PAIRS,
                ins=[rsf_in.opt()], outs=[rsf_out.opt()])

            # final: out_q = x2_q + ffn_q + b2 -> AllGather quad
            ffq = pE.tile([P, KT, LQ], BF16, tag="ffq")
            nc.sync.dma_start(
                ffq[:], rsf_out.rearrange("(o p) t -> p o t", p=P))
            oq = pE.tile([P, KT, LQ], BF16, tag="oq")
            for kt in range(KT):
                nc.vector.tensor_tensor(oq[:, kt, :], x2[:, kt, :],
                                        ffq[:, kt, :], Alu.add)
                nc.vector.tensor_scalar(oq[:, kt, :], oq[:, kt, :], 1.0,
                                        b2s[:, kt:kt + 1], Alu.mult,
                                        Alu.add)
            nc.sync.dma_start(
                ag_in.rearrange("(o p) t -> p o t", p=P), oq[:])
            nc.gpsimd.collective_compute(
                "AllGather", Alu.bypass, QUADS,
                ins=[ag_in.opt()], outs=[ag_out.opt()])
            nc.sync.dma_start(outT[:], ag_out[:])

    nc.compile()
    return nc


_NC_CACHE = None


def _get_nc():
    global _NC_CACHE
    if _NC_CACHE is None:
        _NC_CACHE = build_program()
    return _NC_CACHE


def _q8(x, scale):
    v = np.clip(np.asarray(x, np.float64) * scale, -230.0, 230.0)
    return np.asarray(v).astype(_F8)


def _prep_core(inputs, b, dir_, half):
    hs = slice(half * DIH, (half + 1) * DIH)
    p = "f_" if dir_ == 0 else "b_"
    rank = 2 * dir_ + half
    f32 = np.float32
    m = {}
    xT = np.ascontiguousarray(inputs["x"][b].T.astype(f32))
    xl = xT if dir_ == 0 else np.ascontiguousarray(xT[:, ::-1])
    m["xT_ln"] = xl.astype(_BF)
    m["x_res"] = np.ascontiguousarray(xT[:, rank * LQ:(rank + 1) * LQ])

    W = inputs[p + "in_proj_w"]
    win = np.concatenate(
        [W[hs], W[2 * DIH + half * DIH:2 * DIH + (half + 1) * DIH]],
        axis=0)
    g1 = inputs["norm_g"].astype(np.float64)
    b1n = inputs["norm_b"].astype(np.float64)
    win_eff = win.astype(np.float64) * g1[None, :]
    m["win8"] = np.ascontiguousarray(_q8(win_eff.T, S_W))
    m["win_b"] = (win.astype(np.float64) @ b1n).astype(f32)
    m["sinv"] = np.full((P,), 1.0 / (S_W * S_X), f32)
    m["conv_w"] = np.ascontiguousarray(
        inputs[p + "conv_w"][hs, 0, :]).astype(f32)
    m["conv_b"] = inputs[p + "conv_b"][hs].astype(f32)
    m["a_mat"] = (-np.exp(inputs[p + "A_log"][hs])).astype(f32)
    m["xpw_t"] = np.ascontiguousarray(
        inputs[p + "x_proj_w"][:, hs].T).astype(_BF)
    m["dtw_t"] = np.ascontiguousarray(
        inputs[p + "dt_proj_w"][hs].T).astype(_BF)
    m["dt_b"] = inputs[p + "dt_proj_b"][hs].astype(f32)
    m["d_par"] = inputs[p + "D"][hs].astype(f32)
    m["outw8"] = np.ascontiguousarray(
        _q8(0.5 * inputs[p + "out_proj_w"][:, hs].T, S_O))
    mval = 1.0 / (S_O * S_YG)
    m["mf_in"] = np.full((P,), mval if dir_ == 0 else 0.0, f32)
    m["mr_in"] = np.full((P,), mval if dir_ == 1 else 0.0, f32)
    m["iden_in"] = (np.eye(P) * S_YG).astype(np.float16)

    # FFN: d_ff half by rank parity; the pair shares the L-half
    fh = rank % 2
    fsl = slice(fh * DFH, (fh + 1) * DFH)
    g2 = inputs["ffn_g"].astype(np.float64)
    b2n = inputs["ffn_b"].astype(np.float64)
    w1 = inputs["w1"].astype(np.float64)[fsl]
    m["w1h_t"] = np.ascontiguousarray((w1 * g2[None, :]).T).astype(_BF)
    m["b1_h"] = (inputs["b1"][fsl] + w1 @ b2n).astype(f32)
    m["w2h_t"] = np.ascontiguousarray(inputs["w2"][:, fsl].T).astype(_BF)
    m["b2_e"] = inputs["b2"].astype(f32)
    return m


def make_in_maps(inputs):
    inputs = {k: np.asarray(v) for k, v in inputs.items()}
    maps = []
    for c in range(8):
        b, dir_, half = c // 4, (c // 2) % 2, c % 2
        maps.append(_prep_core(inputs, b, dir_, half))
    return maps


def kernel(**inputs):
    from concourse.bass_utils import run_bass_kernel_spmd
    nc = _get_nc()
    in_maps = make_in_maps(inputs)
    res = run_bass_kernel_spmd(nc, in_maps, core_ids=list(range(8)))
    outs = []
    for c in (0, 4):
        o = res.results[c]["outT"]  # [4, DM, LQ]
        full = np.concatenate([o[q] for q in range(4)], axis=1)
        outs.append(full.T)
    return np.stack(outs).astype(np.float32)
